# revision 1
# baseline (speedup 1.0000x reference)
"""Trainium2 Bass kernel for a transformer decoder layer.

Reference computation (per batch b):
    sa  = causal_attn(y Wq1+bq1, y Wk1+bk1, y Wv1+bv1)      # self-attention
    y1  = LN(y + sa)
    ca  = attn(y1 Wq2+bq2, Z Wk2+bk2, Z Wv2+bv2)            # cross-attention
    y2  = LN(y1 + ca)
    out = LN(y2 + relu(y2 W1 + b1) W2 + b2)
Shapes: B=4, T=S=2048, D=256, F=1024, fp32.

Sharding: 8 cores = 4 batches x 2 parity groups.  Core c=2b+q owns the
rows of batch b with t % 2 == q (1024 rows).  The even/odd interleave
makes the causal-mask work profile identical on both cores of a pair, so
one SPMD program (uniform trip counts) can skip fully-masked key tiles:
query slot J (256 consecutive owned rows = global rows [512J, 512J+512))
only needs keys [0, 512(J+1)).

On-chip layout: scores are computed transposed (keys on partitions,
queries on free dim) so exp(S^T) feeds the P@V matmul directly as the
stationary operand, with no transposes inside the attention loop.
Softmax denominators come for free from an all-ones column appended to V
(padded to D+2 columns to keep matmul free dims even).  Softmax skips
the max-subtraction: scores here are O(5), exp is safe in fp32, and the
additive -1e30 mask still underflows exp to exactly 0.

The host passes y/Z both natural and pre-transposed, so the only
on-device transposes are y1^T / y2^T (PE transposes) which matmul needs
because it contracts over the partition dim.

Matmul operands are bf16 (full PE speed, fast weight load); PSUM
accumulation, softmax, residuals and layernorms all stay fp32.

Host-side bias folding removes three whole device passes:
  - V biases never touch V: softmax rows sum to 1, so attn(V+b) =
    attn(V) + b; bv1 is folded into the y residual rows, bv2 into LN1's
    beta (with bq2 compensated by -bv2 @ Wq2 since y1 feeds Q2).
  - b2 is folded into LN2's beta (with b1 compensated by -b2 @ W1).

target_mask handling: if the mask is exactly causal (tril) the fast
variant above runs.  Any other mask runs a general variant: full key
range for every slot plus a streamed additive transposed mask.
"""

import functools
import sys

import numpy as np

for _p in ("/opt/trn_rl_repo",):
    if _p not in sys.path:
        sys.path.insert(0, _p)

import ml_dtypes  # noqa: E402

import concourse.bass as bass  # noqa: E402
import concourse.tile as tile  # noqa: E402
from concourse import bacc, mybir  # noqa: E402
from concourse.bass_utils import run_bass_kernel_spmd  # noqa: E402
from concourse.masks import make_identity  # noqa: E402

B, T, S, D, F = 4, 2048, 2048, 256, 1024
EPS = 1e-5
NCORES = 8
ROWS = T // 2          # rows per core
NSLOT = 4              # query slots per core
SLOT = ROWS // NSLOT   # 256 query rows per slot
KT = 128               # key tile (partition dim of S^T)
NKS = S // KT          # 16 key tiles total
F32 = mybir.dt.float32
BF16 = mybir.dt.bfloat16
NPBF16 = ml_dtypes.bfloat16
NEG = -1.0e30
ATT_SCALE = 1.0 / np.sqrt(np.float32(D))


def _build_program(causal: bool, apply_g: bool):
    nc = bacc.Bacc("TRN2", target_bir_lowering=False, debug=False,
                   num_devices=NCORES)

    def inp(name, shape, dt=F32):
        return nc.dram_tensor(name, shape, dt, kind="ExternalInput").ap()

    y_rows = inp("y_rows", (ROWS, D))            # residual rows (bv1 folded)
    yT_rows = inp("yT_rows", (D, ROWS), BF16)    # this core's rows, transposed
    yT_full = inp("yT_full", (D, T), BF16)       # full batch seq, transposed
    zT_full = inp("zT_full", (D, S), BF16)
    wq1 = inp("wq1", (D, D), BF16)               # pre-scaled by 1/sqrt(D)
    wk1 = inp("wk1", (D, D), BF16)
    wv1 = inp("wv1", (D, D), BF16)
    wq2 = inp("wq2", (D, D), BF16)               # pre-scaled, bq2 compensated
    wk2 = inp("wk2", (D, D), BF16)
    wv2 = inp("wv2", (D, D), BF16)
    w1 = inp("w1", (D, F), BF16)
    w2 = inp("w2", (F, D), BF16)
    bq1 = inp("bq1", (D,))
    bk1 = inp("bk1", (D,))
    bq2 = inp("bq2", (D,))
    bk2 = inp("bk2", (D,))
    b1 = inp("b1", (F,))
    lng = inp("lng", (3, D))                     # g1,g2,g3
    lnb = inp("lnb", (3, D))                     # be1+bv2, be2+b2, be3
    if causal:
        # additive mask for the diagonal 4-key-tile chunk of each slot,
        # [4, KT, SLOT] in S^T orientation (keys x queries)
        dmask = inp("dmask", (4, KT, SLOT))
    else:
        # full additive mask, S^T orientation: [KT, NKS, ROWS]
        gmask = inp("gmask", (KT, NKS, ROWS))
    out = nc.dram_tensor("out", (ROWS, D), F32, kind="ExternalOutput").ap()

    DO = D // 128   # 2 dout tiles
    DI = D // 128   # 2 din  tiles
    NFT = F // 128  # 8 F tiles
    NTI = ROWS // 128

    def n_keytiles(j):
        return (4 * (j + 1)) if causal else NKS

    with tile.TileContext(nc) as tc:
        with (
            tc.tile_pool(name="const", bufs=1) as constp,
            tc.tile_pool(name="weights", bufs=1) as wp,
            tc.tile_pool(name="acts", bufs=1) as ap_,
            tc.tile_pool(name="small", bufs=4) as smallp,
            tc.tile_pool(name="outp", bufs=3) as outp,
        ):
            # ---- constants / weights ----
            identb = constp.tile([128, 128], BF16)
            make_identity(nc, identb)
            eps_t = constp.tile([128, 1], F32)
            nc.vector.memset(eps_t, EPS)
            # [1, 0] per partition: V's appended softmax-sum / padding cols
            one0_t = constp.tile([128, 2], BF16)
            nc.vector.memset(one0_t[:, 0:1], 1.0)
            nc.vector.memset(one0_t[:, 1:2], 0.0)
            magic_t = constp.tile([128, 2], mybir.dt.int32)
            nc.vector.memset(magic_t, 0x5f3759df)

            def load_w(dram, ko, nfree, name):
                t = wp.tile([128, ko, nfree], BF16, tag=name)
                nc.sync.dma_start(out=t, in_=dram.rearrange(
                    "(o p) n -> p o n", p=128))
                return t

            wq1_s = load_w(wq1, DI, D, "wq1")
            wk1_s = load_w(wk1, DI, D, "wk1")
            wv1_s = load_w(wv1, DI, D, "wv1")
            wq2_s = load_w(wq2, DI, D, "wq2")
            wk2_s = load_w(wk2, DI, D, "wk2")
            wv2_s = load_w(wv2, DI, D, "wv2")
            w1_s = load_w(w1, DI, F, "w1")
            w2_s = load_w(w2, NFT, D, "w2")

            def load_pbias(dram, ko, name):
                # per-partition bias layout [128, ko]
                t = wp.tile([128, ko], F32, tag=name)
                nc.sync.dma_start(out=t, in_=dram.rearrange("(o p) -> p o",
                                                            p=128))
                return t

            bq1_s = load_pbias(bq1, DO, "bq1")
            bk1_s = load_pbias(bk1, DO, "bk1")
            bq2_s = load_pbias(bq2, DO, "bq2")
            bk2_s = load_pbias(bk2, DO, "bk2")
            b1_s = load_pbias(b1, NFT, "b1")

            def load_fbias(dram, name):
                # free-dim vector broadcast to all 128 partitions [128, D]
                t = wp.tile([128, D], F32, tag=name)
                src = bass.AP(tensor=dram.tensor, offset=dram.offset,
                              ap=[[0, 128]] + list(dram.ap))
                nc.sync.dma_start(out=t, in_=src)
                return t

            lng_s = [load_fbias(lng[i], f"lng{i}") for i in range(3)]
            lnb_s = [load_fbias(lnb[i], f"lnb{i}") for i in range(3)]

            if causal:
                dmask_s = constp.tile([128, 4, SLOT], F32)
                nc.sync.dma_start(out=dmask_s, in_=dmask.rearrange(
                    "k p t -> p k t"))

            # ---- persistent activations ----
            y_rows_s = ap_.tile([128, NTI, D], F32, tag="y_rows")
            _yr = y_rows.rearrange("(o p) d -> p o d", p=128)
            for _t0 in range(0, NTI, 2):
                nc.sync.dma_start(out=y_rows_s[:, _t0:_t0 + 2, :],
                                  in_=_yr[:, _t0:_t0 + 2, :])
            y1_s = ap_.tile([128, NTI, D], F32, tag="y1")
            y1T_s = ap_.tile([128, DO, ROWS], BF16, tag="y1T")
            y2_s = ap_.tile([128, NTI, D], F32, tag="y2")
            y2T_s = ap_.tile([128, DO, ROWS], BF16, tag="y2T")

            def rsqrt2(ve):
                """[128, 2] <- 1/sqrt(ve), DVE-only (no ACT table switch).

                Fast-inverse-sqrt seed + 3 Newton steps; ~2e-7 rel err.
                """
                A = mybir.AluOpType
                yi = smallp.tile([128, 2], mybir.dt.int32, tag="rs_i")
                nc.vector.tensor_scalar(out=yi, in0=ve.bitcast(
                    mybir.dt.int32), scalar1=1, scalar2=None,
                    op0=A.logical_shift_right)
                nc.vector.tensor_tensor(out=yi, in0=magic_t, in1=yi,
                                        op=A.subtract)
                y = yi.bitcast(F32)
                h = smallp.tile([128, 2], F32, tag="rs_h")
                nc.vector.tensor_scalar(out=h, in0=ve, scalar1=-0.5,
                                        scalar2=None, op0=A.mult)
                t2 = smallp.tile([128, 2], F32, tag="rs_t")
                for _ in range(2):
                    nc.vector.tensor_mul(out=t2, in0=y, in1=y)
                    nc.vector.tensor_mul(out=t2, in0=h, in1=t2)
                    nc.vector.tensor_scalar(out=t2, in0=t2, scalar1=1.5,
                                            scalar2=None, op0=A.add)
                    nc.vector.tensor_mul(out=y, in0=y, in1=t2)
                return y

            def ln_pair(x_s, gi, outs, t0):
                """outs[ti] = LN(x_s[:, ti, :]) * g[gi] + b[gi] for the
                tile pair (t0, t0+1).  Fully DVE; pipelines per slot."""
                mv = smallp.tile([128, 2, 2], F32, tag="ln_mv")
                for i in range(2):
                    stats = smallp.tile([128, 6], F32, tag="ln_st")
                    nc.vector.bn_stats(out=stats, in_=x_s[:, t0 + i, :])
                    nc.vector.bn_aggr(out=mv[:, i, :], in_=stats)
                ve = smallp.tile([128, 2], F32, tag="ln_ve")
                nc.vector.tensor_scalar(out=ve, in0=mv[:, :, 1],
                                        scalar1=EPS, scalar2=None,
                                        op0=mybir.AluOpType.add)
                rstds = rsqrt2(ve)
                for i in range(2):
                    dst = outs(t0 + i)
                    nc.vector.tensor_scalar(
                        out=dst, in0=x_s[:, t0 + i, :],
                        scalar1=mv[:, i, 0:1],
                        scalar2=rstds[:, i:i + 1],
                        op0=mybir.AluOpType.subtract,
                        op1=mybir.AluOpType.mult)
                    if apply_g:
                        nc.vector.tensor_mul(out=dst, in0=dst, in1=lng_s[gi])
                    nc.vector.tensor_add(out=dst, in0=dst, in1=lnb_s[gi])

            def transpose_to(dst, src_tile, ti, pv_pool):
                """dst[:, dh, ti*128:...] = bf16(src_tile)[:, dh*128:...].T

                PE transpose (bf16, 1 cyc/row); psum borrowed from the PV
                pool (free once the slot's PV has drained)."""
                sb = smallp.tile([128, D], BF16, tag="tp_b")
                nc.vector.tensor_copy(out=sb, in_=src_tile)
                for dh in range(DO):
                    tp = pv_pool.tile([128, 128], BF16, tag="pv")
                    nc.tensor.transpose(tp, sb[:, dh * 128:(dh + 1) * 128],
                                        identb)
                    nc.vector.tensor_copy(
                        out=dst[:, dh, ti * 128:(ti + 1) * 128], in_=tp)

            def project_T(dst, w_s, b_s, src_T, ncols, psum_pool, tag):
                """dst[128, DO, ncols] = (w^T @ src_T) + b  (per-partition b).

                dst = X^T-style layout: partition=dout, free=seq.
                """
                CH = 512
                for do in range(DO):
                    for c0 in range(0, ncols, CH):
                        ps = psum_pool.tile([128, CH], F32, tag=tag)
                        for di in range(DI):
                            nc.tensor.matmul(
                                ps,
                                w_s[:, di, do * 128:(do + 1) * 128],
                                src_T[:, di, c0:c0 + CH],
                                start=(di == 0), stop=(di == DI - 1))
                        nc.scalar.activation(
                            out=dst[:, do, c0:c0 + CH], in_=ps,
                            func=mybir.ActivationFunctionType.Identity,
                            bias=b_s[:, do:do + 1], scale=1.0)

            def project_V(dst, wv_s, src_T, psum_pool, tag):
                """dst[128, NKS, D+2] = rows of (X wv), plus [1, 0] columns
                (col D is the softmax-denominator ones column)."""
                for st in range(NKS):
                    ps = psum_pool.tile([128, D], F32, tag=tag)
                    for di in range(DI):
                        nc.tensor.matmul(
                            ps,
                            src_T[:, di, st * 128:(st + 1) * 128],
                            wv_s[:, di, :],
                            start=(di == 0), stop=(di == DI - 1))
                    nc.scalar.activation(
                        out=dst[:, st, :D], in_=ps,
                        func=mybir.ActivationFunctionType.Copy)
                    nc.vector.tensor_copy(out=dst[:, st, D:D + 2], in_=one0_t)

            def attention(qT_s, kT_s, v_s, resid_s, dst_s, dstT_s, gi,
                          self_attn, psum_pool, pv_pool, ptp, maskp):
                """attn + residual into dst_s, then batched LN + transpose."""
                for j in range(NSLOT):
                    nks = n_keytiles(j) if self_attn else NKS
                    nch = (nks + 3) // 4
                    pt = ptp.tile([128, NKS, SLOT], BF16, tag="pt")
                    for ch in range(nch):
                        st_ps = psum_pool.tile([128, 4, SLOT], F32, tag="st")
                        for i in range(4):
                            ks = ch * 4 + i
                            for di in range(DO):
                                nc.tensor.matmul(
                                    st_ps[:, i, :],
                                    kT_s[:, di, ks * KT:(ks + 1) * KT],
                                    qT_s[:, di, j * SLOT:(j + 1) * SLOT],
                                    start=(di == 0), stop=(di == DO - 1))
                        if self_attn:
                            if causal and ch == nch - 1:
                                nc.vector.tensor_add(out=st_ps, in0=st_ps,
                                                     in1=dmask_s)
                            elif not causal:
                                mk = maskp.tile([128, 4, SLOT], F32,
                                                tag="gmask")
                                nc.sync.dma_start(
                                    out=mk,
                                    in_=gmask[:, ch * 4:ch * 4 + 4,
                                              j * SLOT:(j + 1) * SLOT])
                                nc.vector.tensor_add(out=st_ps, in0=st_ps,
                                                     in1=mk)
                        nc.scalar.activation(
                            out=pt[:, ch * 4:ch * 4 + 4, :], in_=st_ps,
                            func=mybir.ActivationFunctionType.Exp)
                    for th in range(2):
                        ti = 2 * j + th
                        o_ps = pv_pool.tile([128, 512], F32, tag="pv")
                        for ks in range(nks):
                            nc.tensor.matmul(
                                o_ps[:, :D + 2],
                                pt[:, ks, th * 128:(th + 1) * 128],
                                v_s[:, ks, :],
                                start=(ks == 0), stop=(ks == nks - 1))
                        rinv = smallp.tile([128, 1], F32, tag="rinv")
                        nc.vector.reciprocal(out=rinv, in_=o_ps[:, D:D + 1])
                        att = smallp.tile([128, D], F32, tag="att")
                        nc.scalar.activation(
                            out=att, in_=o_ps[:, :D],
                            func=mybir.ActivationFunctionType.Identity,
                            scale=rinv)
                        nc.vector.tensor_add(out=dst_s[:, ti, :], in0=att,
                                             in1=resid_s[:, ti, :])
                    ln_pair(dst_s, gi, lambda ti: dst_s[:, ti, :], 2 * j)
                    for ti in (2 * j, 2 * j + 1):
                        transpose_to(dstT_s, dst_s[:, ti, :], ti, pv_pool)

            # ======== projections (both attentions) + attention ==========
            with (
                tc.tile_pool(name="proj", bufs=1) as prp,
                tc.tile_pool(name="pt", bufs=2) as ptp,
                tc.tile_pool(name="mask1", bufs=2) as mkp1,
                tc.tile_pool(name="ps_mm", bufs=2, space="PSUM") as psMM,
                tc.tile_pool(name="ps_st", bufs=2, space="PSUM") as psST,
                tc.tile_pool(name="ps_pv", bufs=2, space="PSUM") as psPV,
            ):
                k1T_s = prp.tile([128, DO, S], BF16, tag="k1T")
                v1_s = prp.tile([128, NKS, D + 2], BF16, tag="v1")
                q1T_s = prp.tile([128, DO, ROWS], BF16, tag="q1T")
                k2T_s = prp.tile([128, DO, S], BF16, tag="k2T")
                v2_s = prp.tile([128, NKS, D + 2], BF16, tag="v2")
                q2T_s = prp.tile([128, DO, ROWS], BF16, tag="q2T")
                with tc.tile_pool(name="proj_in", bufs=1) as pri:
                    def load_split(dram, tag, width):
                        t = pri.tile([128, DI, width], BF16, tag=tag)
                        rs = dram.rearrange("(o p) t -> p o t", p=128)
                        for c0 in range(0, width, 512):
                            nc.sync.dma_start(out=t[:, :, c0:c0 + 512],
                                              in_=rs[:, :, c0:c0 + 512])
                        return t

                    yT_full_s = load_split(yT_full, "yT_full", T)
                    yT_rows_s = load_split(yT_rows, "yT_rows", ROWS)
                    zT_s = load_split(zT_full, "zT", S)

                    project_T(k1T_s, wk1_s, bk1_s, yT_full_s, S, psMM, "prj")
                    project_T(q1T_s, wq1_s, bq1_s, yT_rows_s, ROWS, psMM,
                              "prj")
                    project_V(v1_s, wv1_s, yT_full_s, psPV, "pv")
                    # cross-attn K/V don't depend on self-attn: emit them
                    # here so the PE can fill self-attention's stall bubbles
                    project_T(k2T_s, wk2_s, bk2_s, zT_s, S, psMM, "prj")
                    project_V(v2_s, wv2_s, zT_s, psPV, "pv")

                    attention(q1T_s, k1T_s, v1_s, y_rows_s, y1_s, y1T_s, 0,
                              True, psST, psPV, ptp, mkp1)

                project_T(q2T_s, wq2_s, bq2_s, y1T_s, ROWS, psMM, "prj")
                attention(q2T_s, k2T_s, v2_s, y1_s, y2_s, y2T_s, 1,
                          False, psST, psPV, ptp, None)

            # ================= phase 5: FFN ===============================
            with (
                tc.tile_pool(name="ffn", bufs=1) as fp_,
                tc.tile_pool(name="ps_e", bufs=3, space="PSUM") as psE,
            ):
                h1T_s = fp_.tile([128, NFT, ROWS], BF16, tag="h1T")
                x3_s = fp_.tile([128, NTI, D], F32, tag="x3")
                CH = 512
                for f in range(NFT):
                    for c0 in range(0, ROWS, CH):
                        ps = psE.tile([128, CH], F32, tag="h1")
                        for di in range(DI):
                            nc.tensor.matmul(
                                ps,
                                w1_s[:, di, f * 128:(f + 1) * 128],
                                y2T_s[:, di, c0:c0 + CH],
                                start=(di == 0), stop=(di == DI - 1))
                        nc.scalar.activation(
                            out=h1T_s[:, f, c0:c0 + CH], in_=ps,
                            func=mybir.ActivationFunctionType.Relu,
                            bias=b1_s[:, f:f + 1], scale=1.0)
                for ti in range(NTI):
                    ps = psE.tile([128, D], F32, tag="ffn2")
                    for f in range(NFT):
                        nc.tensor.matmul(
                            ps,
                            h1T_s[:, f, ti * 128:(ti + 1) * 128],
                            w2_s[:, f, :],
                            start=(f == 0), stop=(f == NFT - 1))
                    nc.vector.tensor_add(out=x3_s[:, ti, :], in0=ps,
                                         in1=y2_s[:, ti, :])
                def emit_out(ti):
                    ot = outp.tile([128, D], F32, tag="out")
                    return ot
                out_tiles = [emit_out(ti) for ti in range(NTI)]
                for t0 in range(0, NTI, 2):
                    ln_pair(x3_s, 2, lambda ti: out_tiles[ti], t0)
                    for ti in (t0, t0 + 1):
                        nc.sync.dma_start(
                            out=out.rearrange("(o p) d -> p o d",
                                              p=128)[:, ti, :],
                            in_=out_tiles[ti])

    nc.compile()
    return nc


@functools.lru_cache(maxsize=4)
def _get_program(causal: bool, apply_g: bool):
    return _build_program(causal, apply_g)


def _is_causal(mask):
    m = np.asarray(mask)
    if m.shape != (T, S):
        return False
    return bool(np.array_equal(m != 0, np.tril(np.ones((T, S), dtype=bool))))


def _make_dmask(q):
    """Additive diag-chunk mask [4, KT, SLOT] (S^T layout) for parity q."""
    ss = np.arange(KT)[:, None]
    tt = np.arange(SLOT)[None, :]
    out = np.empty((4, KT, SLOT), np.float32)
    for i in range(4):
        out[i] = np.where(128 * i + ss <= 2 * tt + q, 0.0, NEG)
    return out


def _make_gmask(mask, q):
    """General additive mask [KT, NKS, ROWS] (S^T layout) for parity q."""
    rows = np.arange(q, T, 2)                      # owned global rows
    mt = np.where(np.asarray(mask)[rows, :] != 0, 0.0, NEG).astype(np.float32)
    # mt is [ROWS(t), S(s)] -> [s, t] -> [KT, NKS, ROWS]
    return np.ascontiguousarray(
        mt.T.reshape(NKS, KT, ROWS).transpose(1, 0, 2))


def _run(y, Z, target_mask, Wq1, bq1, Wk1, bk1, Wv1, bv1,
         Wq2, bq2, Wk2, bk2, Wv2, bv2, W1, b1, W2, b2,
         g1, be1, g2, be2, g3, be3, trace=False, trace_cores=None):
    y = np.ascontiguousarray(np.asarray(y, np.float32))
    Z = np.ascontiguousarray(np.asarray(Z, np.float32))
    f32 = lambda a: np.asarray(a, np.float32)
    bf = lambda a: np.ascontiguousarray(np.asarray(a, np.float32)
                                        .astype(NPBF16))
    causal = _is_causal(target_mask)
    apply_g = not (np.all(f32(g1) == 1) and np.all(f32(g2) == 1)
                   and np.all(f32(g3) == 1))
    nc = _get_program(causal, apply_g)

    # host-side bias folding (see module docstring)
    bq2_adj = (f32(bq2) - f32(bv2) @ f32(Wq2)) * ATT_SCALE
    b1_adj = f32(b1) - f32(b2) @ f32(W1)
    lnb0 = f32(be1) + f32(bv2)
    lnb1 = f32(be2) + f32(b2)

    shared = dict(
        wq1=bf(f32(Wq1) * ATT_SCALE), wk1=bf(Wk1), wv1=bf(Wv1),
        wq2=bf(f32(Wq2) * ATT_SCALE), wk2=bf(Wk2), wv2=bf(Wv2),
        w1=bf(W1), w2=bf(W2),
        bq1=f32(bq1) * ATT_SCALE, bk1=f32(bk1),
        bq2=bq2_adj, bk2=f32(bk2), b1=b1_adj,
        lng=np.stack([f32(g1), f32(g2), f32(g3)]),
        lnb=np.stack([lnb0, lnb1, f32(be3)]),
    )
    bv1f = f32(bv1)
    in_maps = []
    for c in range(NCORES):
        b, q = divmod(c, 2)
        rows = y[b, q::2, :]
        m = dict(shared)
        m["y_rows"] = rows + bv1f
        m["yT_rows"] = bf(rows.T)
        m["yT_full"] = bf(y[b].T)
        m["zT_full"] = bf(Z[b].T)
        if causal:
            m["dmask"] = _make_dmask(q)
        else:
            m["gmask"] = _make_gmask(target_mask, q)
        in_maps.append(m)

    res = run_bass_kernel_spmd(nc, in_maps, core_ids=list(range(NCORES)),
                               trace=trace, trace_cores=trace_cores)
    full = np.empty((B, T, D), np.float32)
    for c in range(NCORES):
        b, q = divmod(c, 2)
        full[b, q::2, :] = res.results[c]["out"]
    return full, res


def kernel(**inputs):
    return _run(**inputs)[0]



# revision 5
# speedup vs baseline: 1.1228x; 1.1228x over previous
"""Trainium2 Bass kernel for a transformer decoder layer.

Reference computation (per batch b):
    sa  = causal_attn(y Wq1+bq1, y Wk1+bk1, y Wv1+bv1)      # self-attention
    y1  = LN(y + sa)
    ca  = attn(y1 Wq2+bq2, Z Wk2+bk2, Z Wv2+bv2)            # cross-attention
    y2  = LN(y1 + ca)
    out = LN(y2 + relu(y2 W1 + b1) W2 + b2)
Shapes: B=4, T=S=2048, D=256, F=1024, fp32.

Sharding: 8 cores = 4 batches x 2 parity groups.  Core c=2b+q owns the
rows of batch b with t % 2 == q (1024 rows).  The even/odd interleave
makes the causal-mask work profile identical on both cores of a pair, so
one SPMD program (uniform trip counts) can skip fully-masked key tiles:
query slot J (256 consecutive owned rows = global rows [512J, 512J+512))
only needs keys [0, 512(J+1)).

On-chip layout: scores are computed transposed (keys on partitions,
queries on free dim) so exp(S^T) feeds the P@V matmul directly as the
stationary operand, with no transposes inside the attention loop.
Softmax denominators come for free from an all-ones column appended to V
(padded to D+2 columns to keep matmul free dims even).  Softmax skips
the max-subtraction: scores here are O(5), exp is safe in fp32, and the
additive -1e30 mask still underflows exp to exactly 0.

Scheduling (the point of this version): every engine queue executes in
emission order, so the program is emitted as one flat software pipeline:
  - input DMAs are emitted in consumption order (wk1/wv1/wq1 -> yT
    chunks -> ...) and K1/V1 projection chunks are emitted per-chunk so
    the PE starts while the rest of the inputs stream in;
  - each attention slot's LN+transpose tail is emitted during the NEXT
    slot's score/PV matmuls, with K2/V2/Q2 projection chunks (self
    phase) and FFN chunks (cross phase) interleaved as PE filler, so the
    PE never sits behind the Vector-side LN chain;
  - the FFN + final LN + output DMA are pipelined per 256-row chunk
    behind cross-attention instead of running as a serial tail.
Other tricks vs the obvious code:
  - LN rstd = exp(-0.5*ln(var+eps)): two tiny Scalar ops (the
    natural_log_exp activation table holds ln AND exp) instead of a
    13-op serial DVE Newton chain;
  - attention PSUM drain is one fused DVE op per tile:
    y1 = (o_ps * 1/denom) + residual  (scalar_tensor_tensor);
  - bias adds and bf16 casts run on the otherwise-idle GpSimd engine.

Matmul operands are bf16 (full PE speed), PSUM accumulation, softmax,
residuals and layernorms stay fp32.

Host-side bias folding removes three whole device passes:
  - V biases never touch V: softmax rows sum to 1, so attn(V+b) =
    attn(V) + b; bv1 is folded into the y residual rows, bv2 into LN1's
    beta (with bq2 compensated by -bv2 @ Wq2 since y1 feeds Q2).
  - b2 is folded into LN2's beta (with b1 compensated by -b2 @ W1).

target_mask handling: if the mask is exactly causal (tril) the fast
variant above runs.  Any other mask runs a general variant: full key
range for every slot plus a streamed additive transposed mask.
"""

import functools
import sys

import numpy as np

for _p in ("/opt/trn_rl_repo",):
    if _p not in sys.path:
        sys.path.insert(0, _p)

import ml_dtypes  # noqa: E402

import concourse.bass as bass  # noqa: E402
import concourse.tile as tile  # noqa: E402
from concourse import bacc, mybir  # noqa: E402
from concourse.bass_utils import run_bass_kernel_spmd  # noqa: E402
from concourse.masks import make_identity  # noqa: E402

B, T, S, D, F = 4, 2048, 2048, 256, 1024
EPS = 1e-5
NCORES = 8
ROWS = T // 2          # rows per core
NSLOT = 4              # query slots per core
SLOT = ROWS // NSLOT   # 256 query rows per slot
KT = 128               # key tile (partition dim of S^T)
NKS = S // KT          # 16 key tiles total
F32 = mybir.dt.float32
BF16 = mybir.dt.bfloat16
NPBF16 = ml_dtypes.bfloat16
NEG = -1.0e30
ATT_SCALE = 1.0 / np.sqrt(np.float32(D))
A = mybir.AluOpType
AF = mybir.ActivationFunctionType


def _build_program(causal: bool, apply_g: bool, zb3: bool):
    nc = bacc.Bacc("TRN2", target_bir_lowering=False, debug=False,
                   num_devices=NCORES)

    def inp(name, shape, dt=F32):
        return nc.dram_tensor(name, shape, dt, kind="ExternalInput").ap()

    y_rows = inp("y_rows", (ROWS, D))            # residual rows (bv1 folded)
    yT_rows = inp("yT_rows", (D, ROWS), BF16)    # this core's rows, transposed
    yT_full = inp("yT_full", (D, T), BF16)       # full batch seq, transposed
    zT_full = inp("zT_full", (D, S), BF16)
    wq1 = inp("wq1", (D, D), BF16)               # pre-scaled by 1/sqrt(D)
    wk1 = inp("wk1", (D, D), BF16)
    wv1 = inp("wv1", (D, D), BF16)
    wq2 = inp("wq2", (D, D), BF16)               # pre-scaled, bq2 compensated
    wk2 = inp("wk2", (D, D), BF16)
    wv2 = inp("wv2", (D, D), BF16)
    w1 = inp("w1", (D, F), BF16)
    w2 = inp("w2", (F, D), BF16)
    bq1 = inp("bq1", (D,))
    bk1 = inp("bk1", (D,))
    bq2 = inp("bq2", (D,))
    bk2 = inp("bk2", (D,))
    b1 = inp("b1", (F,))
    lng = inp("lng", (3, D))                     # g1,g2,g3
    lnb = inp("lnb", (3, D))                     # be1+bv2, be2+b2, be3
    if causal:
        # additive mask for the diagonal 4-key-tile chunk of each slot,
        # [4, KT, SLOT] in S^T orientation (keys x queries)
        dmask = inp("dmask", (4, KT, SLOT))
    else:
        # full additive mask, S^T orientation: [KT, NKS, ROWS]
        gmask = inp("gmask", (KT, NKS, ROWS))
    out = nc.dram_tensor("out", (ROWS, D), F32, kind="ExternalOutput").ap()

    DO = D // 128   # 2 dout tiles
    DI = D // 128   # 2 din  tiles
    NFT = F // 128  # 8 F tiles
    NTI = ROWS // 128
    NCH = T // 512  # 4 column chunks of the full sequence

    def n_keytiles(j):
        return (4 * (j + 1)) if causal else NKS

    with tile.TileContext(nc) as tc:
        with (
            tc.tile_pool(name="const", bufs=1) as constp,
            tc.tile_pool(name="weights", bufs=1) as wp,
            tc.tile_pool(name="acts", bufs=1) as ap_,
            tc.tile_pool(name="small", bufs=4) as smallp,
            tc.tile_pool(name="outp", bufs=3) as outp,
            tc.tile_pool(name="pt", bufs=2) as ptp,
            tc.tile_pool(name="mask1", bufs=2) as mkp1,
            tc.tile_pool(name="ps_st", bufs=2, space="PSUM") as psST,
            tc.tile_pool(name="ps_pv", bufs=2, space="PSUM") as psPV,
            tc.tile_pool(name="ps_mm", bufs=2, space="PSUM") as psMM,
        ):
            # ---- constants ----
            identb = constp.tile([128, 128], BF16)
            make_identity(nc, identb)
            one0_t = constp.tile([128, 2], BF16)
            nc.vector.memset(one0_t[:, 0:1], 1.0)
            nc.vector.memset(one0_t[:, 1:2], 0.0)
            eps_t = constp.tile([128, 1], F32)
            nc.vector.memset(eps_t, EPS)
            nhalf_t = constp.tile([128, 1], F32)
            nc.vector.memset(nhalf_t, -0.5)

            # ---- input DMAs, emitted in consumption order ----
            def load_w(dram, ko, nfree, name):
                t = wp.tile([128, ko, nfree], BF16, tag=name)
                nc.sync.dma_start(out=t, in_=dram.rearrange(
                    "(o p) n -> p o n", p=128))
                return t

            def load_pbias(dram, ko, name):
                # per-partition bias layout [128, ko]
                t = wp.tile([128, ko], F32, tag=name)
                nc.sync.dma_start(out=t, in_=dram.rearrange("(o p) -> p o",
                                                            p=128))
                return t

            def load_fbias(dram, name):
                # free-dim vector broadcast to all 128 partitions [128, D]
                t = wp.tile([128, D], F32, tag=name)
                src = bass.AP(tensor=dram.tensor, offset=dram.offset,
                              ap=[[0, 128]] + list(dram.ap))
                nc.sync.dma_start(out=t, in_=src)
                return t

            def load_split(pool, dram, tag, width):
                t = pool.tile([128, DI, width], BF16, tag=tag)
                rs = dram.rearrange("(o p) t -> p o t", p=128)
                for c0 in range(0, width, 512):
                    nc.sync.dma_start(out=t[:, :, c0:c0 + 512],
                                      in_=rs[:, :, c0:c0 + 512])
                return t

            wk1_s = load_w(wk1, DI, D, "wk1")
            wv1_s = load_w(wv1, DI, D, "wv1")
            wq1_s = load_w(wq1, DI, D, "wq1")
            bk1_s = load_pbias(bk1, DO, "bk1")
            bq1_s = load_pbias(bq1, DO, "bq1")
            yT_full_s = load_split(ap_, yT_full, "yT_full", T)
            yT_rows_s = load_split(ap_, yT_rows, "yT_rows", ROWS)
            if causal:
                dmask_s = constp.tile([128, 4, SLOT], F32)
                nc.sync.dma_start(out=dmask_s, in_=dmask.rearrange(
                    "k p t -> p k t"))
            y_rows_s = ap_.tile([128, NTI, D], F32, tag="y_rows")
            _yr = y_rows.rearrange("(o p) d -> p o d", p=128)
            for _t0 in range(0, NTI, 2):
                nc.sync.dma_start(out=y_rows_s[:, _t0:_t0 + 2, :],
                                  in_=_yr[:, _t0:_t0 + 2, :])
            wk2_s = load_w(wk2, DI, D, "wk2")
            wv2_s = load_w(wv2, DI, D, "wv2")
            bk2_s = load_pbias(bk2, DO, "bk2")
            zT_s = load_split(ap_, zT_full, "zT", S)
            lnb_s = [load_fbias(lnb[i], f"lnb{i}")
                     for i in range(2 if zb3 else 3)]
            if apply_g:
                lng_s = [load_fbias(lng[i], f"lng{i}") for i in range(3)]
            wq2_s = load_w(wq2, DI, D, "wq2")
            bq2_s = load_pbias(bq2, DO, "bq2")
            w1_s = load_w(w1, DI, F, "w1")
            b1_s = load_pbias(b1, NFT, "b1")
            w2_s = load_w(w2, NFT, D, "w2")

            # ---- persistent activations ----
            y1_s = ap_.tile([128, NTI, D], F32, tag="y1")
            y1T_s = ap_.tile([128, DO, ROWS], BF16, tag="y1T")
            y2_s = ap_.tile([128, NTI, D], F32, tag="y2")
            y2T_s = ap_.tile([128, DO, ROWS], BF16, tag="y2T")
            x3_s = ap_.tile([128, NTI, D], F32, tag="x3")
            h1T_s = ap_.tile([128, NFT, ROWS], BF16, tag="h1T")
            k1T_s = ap_.tile([128, DO, S], BF16, tag="k1T")
            v1_s = ap_.tile([128, NKS, D + 2], BF16, tag="v1")
            q1T_s = ap_.tile([128, DO, ROWS], BF16, tag="q1T")
            k2T_s = ap_.tile([128, DO, S], BF16, tag="k2T")
            v2_s = ap_.tile([128, NKS, D + 2], BF16, tag="v2")
            q2T_s = ap_.tile([128, DO, ROWS], BF16, tag="q2T")

            # ones / zero pad columns of V (softmax denominator trick):
            # one strided memset covers all key tiles
            for v_s in (v1_s, v2_s):
                nc.vector.memset(v_s[:, :, D:D + 1], 1.0)
                nc.vector.memset(v_s[:, :, D + 1:D + 2], 0.0)

            # ---- emission helpers ----
            def proj_unit(dst, w_s, b_s, src_T, do, c0, cw):
                """dst[:, do, c0:c0+cw] = (w^T @ src_T)[:, c0:] + b."""
                ps = psMM.tile([128, 512], F32, tag="prj")
                for di in range(DI):
                    nc.tensor.matmul(
                        ps[:, :cw],
                        w_s[:, di, do * 128:(do + 1) * 128],
                        src_T[:, di, c0:c0 + cw],
                        start=(di == 0), stop=(di == DI - 1))
                nc.scalar.activation(
                    out=dst[:, do, c0:c0 + cw], in_=ps[:, :cw],
                    func=AF.Identity, bias=b_s[:, do:do + 1], scale=1.0)

            def projV_unit(dst, wv_s, src_T, c):
                """dst[:, 4c:4c+4, :D] = rows of (X wv) for 4 key tiles."""
                for st in range(4 * c, 4 * c + 4):
                    ps = psMM.tile([128, 512], F32, tag="prj")
                    for di in range(DI):
                        nc.tensor.matmul(
                            ps[:, :D],
                            src_T[:, di, st * 128:(st + 1) * 128],
                            wv_s[:, di, :],
                            start=(di == 0), stop=(di == DI - 1))
                    nc.scalar.activation(
                        out=dst[:, st, :D], in_=ps[:, :D], func=AF.Copy)

            def emit_scores(j, kT_s, qT_s, self_attn):
                """exp(K^T Q + mask) for slot j -> pt tile (bf16)."""
                nks = n_keytiles(j) if self_attn else NKS
                nch = (nks + 3) // 4
                pt = ptp.tile([128, NKS, SLOT], BF16, tag="pt")
                for ch in range(nch):
                    st_ps = psST.tile([128, 4, SLOT], F32, tag="st")
                    for i in range(4):
                        ks = ch * 4 + i
                        for di in range(DO):
                            nc.tensor.matmul(
                                st_ps[:, i, :],
                                kT_s[:, di, ks * KT:(ks + 1) * KT],
                                qT_s[:, di, j * SLOT:(j + 1) * SLOT],
                                start=(di == 0), stop=(di == DO - 1))
                    if self_attn:
                        if causal and ch == nch - 1:
                            nc.vector.tensor_add(out=st_ps, in0=st_ps,
                                                 in1=dmask_s)
                        elif not causal:
                            mk = mkp1.tile([128, 4, SLOT], F32, tag="gmask")
                            nc.sync.dma_start(
                                out=mk,
                                in_=gmask[:, ch * 4:ch * 4 + 4,
                                          j * SLOT:(j + 1) * SLOT])
                            nc.vector.tensor_add(out=st_ps, in0=st_ps,
                                                 in1=mk)
                    nc.scalar.activation(
                        out=pt[:, ch * 4:ch * 4 + 4, :], in_=st_ps,
                        func=AF.Exp)
                return pt, nks

            def emit_pv(j, pt, nks, v_s):
                """P @ V for both 128-row tiles of slot j -> psum pair."""
                o_pair = []
                for th in range(2):
                    o_ps = psPV.tile([128, 512], F32, tag="pv")
                    for ks in range(nks):
                        nc.tensor.matmul(
                            o_ps[:, :D + 2],
                            pt[:, ks, th * 128:(th + 1) * 128],
                            v_s[:, ks, :],
                            start=(ks == 0), stop=(ks == nks - 1))
                    o_pair.append(o_ps)
                return o_pair

            def emit_drain(j, o_pair, resid_s, dst_s):
                """dst = o/denom + resid, one fused DVE op per tile."""
                for th in range(2):
                    ti = 2 * j + th
                    o_ps = o_pair[th]
                    rinv = smallp.tile([128, 1], F32, tag="rinv")
                    nc.vector.reciprocal(out=rinv, in_=o_ps[:, D:D + 1])
                    nc.vector.scalar_tensor_tensor(
                        out=dst_s[:, ti, :], in0=o_ps[:, :D], scalar=rinv,
                        in1=resid_s[:, ti, :], op0=A.mult, op1=A.add)

            def emit_ln(x_s, gi, outs, t0, cast_to=None, ti_base=None):
                """outs(ti) = LN(x_s[:, ti, :]) for the pair (t0, t0+1).

                rstd = exp(-0.5*ln(var+eps)) on Scalar; bias add + bf16
                cast offloaded to GpSimd.  If cast_to is given, also
                emit PE transposes of the pair into cast_to (y*T)."""
                mv = smallp.tile([128, 2, 2], F32, tag="ln_mv")
                for i in range(2):
                    stats = smallp.tile([128, 6], F32, tag="ln_st")
                    nc.vector.bn_stats(out=stats, in_=x_s[:, t0 + i, :])
                    nc.vector.bn_aggr(out=mv[:, i, :], in_=stats)
                lnv = smallp.tile([128, 2], F32, tag="ln_lv")
                nc.scalar.activation(out=lnv, in_=mv[:, :, 1],
                                     func=AF.Ln, bias=eps_t, scale=1.0)
                rstd = smallp.tile([128, 2], F32, tag="ln_rs")
                nc.scalar.activation(out=rstd, in_=lnv,
                                     func=AF.Exp, bias=0.0, scale=nhalf_t)
                use_b = (gi < 2) or not zb3
                for i in range(2):
                    dst = outs(t0 + i)
                    nc.vector.tensor_scalar(
                        out=dst, in0=x_s[:, t0 + i, :],
                        scalar1=mv[:, i, 0:1],
                        scalar2=rstd[:, i:i + 1],
                        op0=A.subtract,
                        op1=A.mult)
                    if apply_g:
                        nc.vector.tensor_mul(out=dst, in0=dst, in1=lng_s[gi])
                    if use_b:
                        nc.gpsimd.tensor_add(out=dst, in0=dst, in1=lnb_s[gi])
                if cast_to is not None:
                    for i in range(2):
                        ti = t0 + i if ti_base is None else ti_base + i
                        sb = smallp.tile([128, D], BF16, tag="tp_b")
                        nc.gpsimd.tensor_copy(out=sb, in_=outs(t0 + i))
                        for dh in range(DO):
                            tp = psPV.tile([128, 128], BF16, tag="pv")
                            nc.tensor.transpose(
                                tp, sb[:, dh * 128:(dh + 1) * 128], identb)
                            nc.vector.tensor_copy(
                                out=cast_to[:, dh, ti * 128:(ti + 1) * 128],
                                in_=tp)

            def attn_tail(j, dst_s, dstT_s, gi):
                emit_ln(dst_s, gi, lambda ti: dst_s[:, ti, :], 2 * j,
                        cast_to=dstT_s)

            def emit_ffn1(c):
                """h1T chunk c (256 cols) for all F tiles, relu+bias."""
                cols = slice(c * SLOT, (c + 1) * SLOT)
                for f0 in range(0, NFT, 2):
                    ps = psMM.tile([128, 2, SLOT], F32, tag="prj")
                    for fi in range(2):
                        f = f0 + fi
                        for di in range(DI):
                            nc.tensor.matmul(
                                ps[:, fi, :],
                                w1_s[:, di, f * 128:(f + 1) * 128],
                                y2T_s[:, di, cols],
                                start=(di == 0), stop=(di == DI - 1))
                    for fi in range(2):
                        f = f0 + fi
                        nc.scalar.activation(
                            out=h1T_s[:, f, cols], in_=ps[:, fi, :],
                            func=AF.Relu, bias=b1_s[:, f:f + 1], scale=1.0)

            def emit_ffn2(c):
                """x3 tiles (2c, 2c+1): ffn2 matmul + residual add."""
                for th in range(2):
                    ti = 2 * c + th
                    ps = psMM.tile([128, 512], F32, tag="prj")
                    for f in range(NFT):
                        nc.tensor.matmul(
                            ps[:, :D],
                            h1T_s[:, f, ti * 128:(ti + 1) * 128],
                            w2_s[:, f, :],
                            start=(f == 0), stop=(f == NFT - 1))
                    nc.vector.tensor_add(out=x3_s[:, ti, :], in0=ps[:, :D],
                                         in1=y2_s[:, ti, :])

            def emit_out(p):
                """LN3 + output DMA for tile pair p."""
                t0 = 2 * p
                o_tiles = {}

                def ot(ti):
                    if ti not in o_tiles:
                        t = outp.tile([128, D], F32, tag="out")
                        o_tiles[ti] = t
                    return o_tiles[ti]

                emit_ln(x3_s, 2, ot, t0)
                for ti in (t0, t0 + 1):
                    nc.sync.dma_start(
                        out=out.rearrange("(o p) d -> p o d",
                                          p=128)[:, ti, :],
                        in_=ot(ti))

            # ================= emission schedule ======================
            # -- startup: K1/V1 per chunk as yT chunks land, then Q1 --
            for c in range(NCH):
                for do in range(DO):
                    proj_unit(k1T_s, wk1_s, bk1_s, yT_full_s, do, c * 512,
                              512)
                projV_unit(v1_s, wv1_s, yT_full_s, c)
            for do in range(DO):
                for c0 in (0, 512):
                    proj_unit(q1T_s, wq1_s, bq1_s, yT_rows_s, do, c0, 512)

            # -- self-attention slots, pipelined --
            # fillers: k2/v2 projection units spread across slots 1..3
            fillers = []
            for c in range(NCH):
                fillers.append(lambda c=c: [
                    proj_unit(k2T_s, wk2_s, bk2_s, zT_s, do, c * 512, 512)
                    for do in range(DO)])
                fillers.append(lambda c=c: projV_unit(v2_s, wv2_s, zT_s, c))
            fill_plan = {0: 2, 1: 2, 2: 2, 3: 1}  # units after each slot
            for j in range(NSLOT):
                pt, nks = emit_scores(j, k1T_s, q1T_s, True)
                o_pair = emit_pv(j, pt, nks, v1_s)
                if j > 0:
                    attn_tail(j - 1, y1_s, y1T_s, 0)
                emit_drain(j, o_pair, y_rows_s, y1_s)
                for _ in range(fill_plan[j]):
                    fillers.pop(0)()
            while fillers:
                fillers.pop(0)()
            attn_tail(NSLOT - 1, y1_s, y1T_s, 0)
            for do in range(DO):
                for c0 in (0, 512):
                    proj_unit(q2T_s, wq2_s, bq2_s, y1T_s, do, c0, 512)

            # -- cross-attention slots + FFN + LN3 + out, pipelined --
            for j in range(NSLOT):
                pt, nks = emit_scores(j, k2T_s, q2T_s, False)
                o_pair = emit_pv(j, pt, nks, v2_s)
                if j > 0:
                    attn_tail(j - 1, y2_s, y2T_s, 1)
                emit_drain(j, o_pair, y1_s, y2_s)
                if j >= 2:
                    emit_ffn1(j - 2)
                    emit_ffn2(j - 2)
                if j >= 3:
                    emit_out(j - 3)
            attn_tail(NSLOT - 1, y2_s, y2T_s, 1)
            emit_ffn1(2)
            emit_ffn2(2)
            emit_out(1)
            emit_ffn1(3)
            emit_ffn2(3)
            emit_out(2)
            emit_out(3)

    nc.compile()
    return nc


@functools.lru_cache(maxsize=4)
def _get_program(causal: bool, apply_g: bool, zb3: bool):
    return _build_program(causal, apply_g, zb3)


def _is_causal(mask):
    m = np.asarray(mask)
    if m.shape != (T, S):
        return False
    return bool(np.array_equal(m != 0, np.tril(np.ones((T, S), dtype=bool))))


def _make_dmask(q):
    """Additive diag-chunk mask [4, KT, SLOT] (S^T layout) for parity q."""
    ss = np.arange(KT)[:, None]
    tt = np.arange(SLOT)[None, :]
    out = np.empty((4, KT, SLOT), np.float32)
    for i in range(4):
        out[i] = np.where(128 * i + ss <= 2 * tt + q, 0.0, NEG)
    return out


def _make_gmask(mask, q):
    """General additive mask [KT, NKS, ROWS] (S^T layout) for parity q."""
    rows = np.arange(q, T, 2)                      # owned global rows
    mt = np.where(np.asarray(mask)[rows, :] != 0, 0.0, NEG).astype(np.float32)
    # mt is [ROWS(t), S(s)] -> [s, t] -> [KT, NKS, ROWS]
    return np.ascontiguousarray(
        mt.T.reshape(NKS, KT, ROWS).transpose(1, 0, 2))


def _run(y, Z, target_mask, Wq1, bq1, Wk1, bk1, Wv1, bv1,
         Wq2, bq2, Wk2, bk2, Wv2, bv2, W1, b1, W2, b2,
         g1, be1, g2, be2, g3, be3, trace=False, trace_cores=None):
    y = np.ascontiguousarray(np.asarray(y, np.float32))
    Z = np.ascontiguousarray(np.asarray(Z, np.float32))
    f32 = lambda a: np.asarray(a, np.float32)
    bf = lambda a: np.ascontiguousarray(np.asarray(a, np.float32)
                                        .astype(NPBF16))
    causal = _is_causal(target_mask)
    apply_g = not (np.all(f32(g1) == 1) and np.all(f32(g2) == 1)
                   and np.all(f32(g3) == 1))
    zb3 = bool(np.all(f32(be3) == 0))
    nc = _get_program(causal, apply_g, zb3)

    # host-side bias folding (see module docstring)
    bq2_adj = (f32(bq2) - f32(bv2) @ f32(Wq2)) * ATT_SCALE
    b1_adj = f32(b1) - f32(b2) @ f32(W1)
    lnb0 = f32(be1) + f32(bv2)
    lnb1 = f32(be2) + f32(b2)

    shared = dict(
        wq1=bf(f32(Wq1) * ATT_SCALE), wk1=bf(Wk1), wv1=bf(Wv1),
        wq2=bf(f32(Wq2) * ATT_SCALE), wk2=bf(Wk2), wv2=bf(Wv2),
        w1=bf(W1), w2=bf(W2),
        bq1=f32(bq1) * ATT_SCALE, bk1=f32(bk1),
        bq2=bq2_adj, bk2=f32(bk2), b1=b1_adj,
        lng=np.stack([f32(g1), f32(g2), f32(g3)]),
        lnb=np.stack([lnb0, lnb1, f32(be3)]),
    )
    bv1f = f32(bv1)
    in_maps = []
    for c in range(NCORES):
        b, q = divmod(c, 2)
        rows = y[b, q::2, :]
        m = dict(shared)
        m["y_rows"] = rows + bv1f
        m["yT_rows"] = bf(rows.T)
        m["yT_full"] = bf(y[b].T)
        m["zT_full"] = bf(Z[b].T)
        if causal:
            m["dmask"] = _make_dmask(q)
        else:
            m["gmask"] = _make_gmask(target_mask, q)
        in_maps.append(m)

    res = run_bass_kernel_spmd(nc, in_maps, core_ids=list(range(NCORES)),
                               trace=trace, trace_cores=trace_cores)
    full = np.empty((B, T, D), np.float32)
    for c in range(NCORES):
        b, q = divmod(c, 2)
        full[b, q::2, :] = res.results[c]["out"]
    return full, res


def kernel(**inputs):
    return _run(**inputs)[0]


# revision 11
# speedup vs baseline: 1.4930x; 1.3297x over previous
"""Trainium2 Bass kernel for a transformer decoder layer.

Reference computation (per batch b):
    sa  = causal_attn(y Wq1+bq1, y Wk1+bk1, y Wv1+bv1)      # self-attention
    y1  = LN(y + sa)
    ca  = attn(y1 Wq2+bq2, Z Wk2+bk2, Z Wv2+bv2)            # cross-attention
    y2  = LN(y1 + ca)
    out = LN(y2 + relu(y2 W1 + b1) W2 + b2)
Shapes: B=4, T=S=2048, D=256, F=1024, fp32.

Sharding: 8 cores = 4 batches x 2 parity groups.  Core c=2b+q owns the
rows of batch b with t % 2 == q (1024 rows).  The even/odd interleave
makes the causal-mask work profile identical on both cores of a pair, so
one SPMD program (uniform trip counts) can skip fully-masked key tiles:
query slot J (256 consecutive owned rows = global rows [512J, 512J+512))
only needs keys [0, 512(J+1)).

On-chip layout: scores are computed transposed (keys on partitions,
queries on free dim) so exp(S^T) feeds the P@V matmul directly as the
stationary operand, with no transposes inside the attention loop.
Softmax denominators come for free from an all-ones column appended to V
(padded to D+2 columns to keep matmul free dims even).  Softmax skips
the max-subtraction: scores here are O(5), exp is safe in fp32, and the
additive -1e30 mask still underflows exp to exactly 0.

Scheduling (the point of this version): every engine queue executes in
emission order, so the program is emitted as one flat software pipeline:
  - input DMAs are emitted in consumption order (wk1/wv1/wq1 -> yT
    chunks -> ...) and K1/V1 projection chunks are emitted per-chunk so
    the PE starts while the rest of the inputs stream in;
  - each attention slot's LN+transpose tail is emitted during the NEXT
    slot's score/PV matmuls, with K2/V2/Q2 projection chunks (self
    phase) and FFN chunks (cross phase) interleaved as PE filler, so the
    PE never sits behind the Vector-side LN chain;
  - the FFN + final LN + output DMA are pipelined per 256-row chunk
    behind cross-attention instead of running as a serial tail.
Other tricks vs the obvious code:
  - LN rstd = exp(-0.5*ln(var+eps)): two tiny Scalar ops (the
    natural_log_exp activation table holds ln AND exp) instead of a
    13-op serial DVE Newton chain;
  - attention PSUM drain is one fused DVE op per tile:
    y1 = (o_ps * 1/denom) + residual  (scalar_tensor_tensor);
  - bias adds and bf16 casts run on the otherwise-idle GpSimd engine.

Matmul operands are bf16 (full PE speed), PSUM accumulation, softmax,
residuals and layernorms stay fp32.

Host-side bias folding removes three whole device passes:
  - V biases never touch V: softmax rows sum to 1, so attn(V+b) =
    attn(V) + b; bv1 is folded into the y residual rows, bv2 into LN1's
    beta (with bq2 compensated by -bv2 @ Wq2 since y1 feeds Q2).
  - b2 is folded into LN2's beta (with b1 compensated by -b2 @ W1).

target_mask handling: if the mask is exactly causal (tril) the fast
variant above runs.  Any other mask runs a general variant: full key
range for every slot plus a streamed additive transposed mask.
"""

import functools
import sys

import numpy as np

for _p in ("/opt/trn_rl_repo",):
    if _p not in sys.path:
        sys.path.insert(0, _p)

import ml_dtypes  # noqa: E402

import concourse.bass as bass  # noqa: E402
import concourse.tile as tile  # noqa: E402
from concourse import bacc, mybir  # noqa: E402
from concourse.bass_utils import run_bass_kernel_spmd  # noqa: E402
from concourse.masks import make_identity  # noqa: E402

B, T, S, D, F = 4, 2048, 2048, 256, 1024
EPS = 1e-5
NCORES = 8
ROWS = T // 2          # rows per core
NSLOT = 4              # query slots per core
SLOT = ROWS // NSLOT   # 256 query rows per slot
KT = 128               # key tile (partition dim of S^T)
NKS = S // KT          # 16 key tiles total
F32 = mybir.dt.float32
BF16 = mybir.dt.bfloat16
NPBF16 = ml_dtypes.bfloat16
NEG = -1.0e30
ATT_SCALE = 1.0 / np.sqrt(np.float32(D))
A = mybir.AluOpType
AF = mybir.ActivationFunctionType


def _build_program(causal: bool, apply_g: bool, zb3: bool):
    nc = bacc.Bacc("TRN2", target_bir_lowering=False, debug=False,
                   num_devices=NCORES)

    def inp(name, shape, dt=F32):
        return nc.dram_tensor(name, shape, dt, kind="ExternalInput").ap()

    y_rows = inp("y_rows", (ROWS, D))            # residual rows (bv1 folded)
    yT_rows = inp("yT_rows", (D, ROWS), BF16)    # this core's rows, transposed
    yT_full = inp("yT_full", (D, T), BF16)       # full batch seq, transposed
    zT_full = inp("zT_full", (D, S), BF16)
    wq1 = inp("wq1", (D, D), BF16)               # pre-scaled by 1/sqrt(D)
    wk1 = inp("wk1", (D, D), BF16)
    wv1 = inp("wv1", (D, D), BF16)
    wq2 = inp("wq2", (D, D), BF16)               # pre-scaled, bq2 compensated
    wk2 = inp("wk2", (D, D), BF16)
    wv2 = inp("wv2", (D, D), BF16)
    w1 = inp("w1", (D, F), BF16)
    w2 = inp("w2", (F, D), BF16)
    bq1 = inp("bq1", (D,))
    bk1 = inp("bk1", (D,))
    bq2 = inp("bq2", (D,))
    bk2 = inp("bk2", (D,))
    b1 = inp("b1", (F,))
    lng = inp("lng", (3, D))                     # g1,g2,g3
    lnb = inp("lnb", (3, D))                     # be1+bv2, be2+b2, be3
    if causal:
        # additive mask for the diagonal 4-key-tile chunk of each slot,
        # [4, KT, SLOT] in S^T orientation (keys x queries)
        dmask = inp("dmask", (4, KT, SLOT))
    else:
        # full additive mask, S^T orientation: [KT, NKS, ROWS]
        gmask = inp("gmask", (KT, NKS, ROWS))
    out = nc.dram_tensor("out", (ROWS, D), F32, kind="ExternalOutput").ap()

    DO = D // 128   # 2 dout tiles
    DI = D // 128   # 2 din  tiles
    NFT = F // 128  # 8 F tiles
    NTI = ROWS // 128
    NCH = T // 512  # 4 column chunks of the full sequence

    def n_keytiles(j):
        return (4 * (j + 1)) if causal else NKS

    with tile.TileContext(nc) as tc:
        with (
            tc.tile_pool(name="const", bufs=1) as constp,
            tc.tile_pool(name="weights", bufs=1) as wp,
            tc.tile_pool(name="acts", bufs=1) as ap_,
            tc.tile_pool(name="small", bufs=4) as smallp,
            tc.tile_pool(name="outp", bufs=3) as outp,
            tc.tile_pool(name="pt", bufs=2) as ptp,
            tc.tile_pool(name="mask1", bufs=2) as mkp1,
            tc.tile_pool(name="ps_st", bufs=2, space="PSUM") as psST,
            tc.tile_pool(name="ps_pv", bufs=2, space="PSUM") as psPV,
            tc.tile_pool(name="ps_mm", bufs=2, space="PSUM") as psMM,
        ):
            # ---- constants ----
            identb = constp.tile([128, 128], BF16)
            make_identity(nc, identb)
            one0_t = constp.tile([128, 2], BF16)
            nc.vector.memset(one0_t[:, 0:1], 1.0)
            nc.vector.memset(one0_t[:, 1:2], 0.0)
            magic_t = constp.tile([128, 2], mybir.dt.int32)
            nc.vector.memset(magic_t, 0x5f3759df)

            # ---- input DMAs, emitted in consumption order ----
            def load_w(dram, ko, nfree, name):
                t = wp.tile([128, ko, nfree], BF16, tag=name)
                nc.sync.dma_start(out=t, in_=dram.rearrange(
                    "(o p) n -> p o n", p=128))
                return t

            def load_pbias(dram, ko, name):
                # per-partition bias layout [128, ko]
                t = wp.tile([128, ko], F32, tag=name)
                nc.sync.dma_start(out=t, in_=dram.rearrange("(o p) -> p o",
                                                            p=128))
                return t

            def load_fbias(dram, name):
                # free-dim vector broadcast to all 128 partitions [128, D]
                t = wp.tile([128, D], F32, tag=name)
                src = bass.AP(tensor=dram.tensor, offset=dram.offset,
                              ap=[[0, 128]] + list(dram.ap))
                nc.sync.dma_start(out=t, in_=src)
                return t

            def load_split(pool, dram, tag, width):
                t = pool.tile([128, DI, width], BF16, tag=tag)
                rs = dram.rearrange("(o p) t -> p o t", p=128)
                for c0 in range(0, width, 512):
                    nc.sync.dma_start(out=t[:, :, c0:c0 + 512],
                                      in_=rs[:, :, c0:c0 + 512])
                return t

            wk1_s = load_w(wk1, DI, D, "wk1")
            wv1_s = load_w(wv1, DI, D, "wv1")
            wq1_s = load_w(wq1, DI, D, "wq1")
            bk1_s = load_pbias(bk1, DO, "bk1")
            bq1_s = load_pbias(bq1, DO, "bq1")
            yT_full_s = load_split(ap_, yT_full, "yT_full", T)
            yT_rows_s = load_split(ap_, yT_rows, "yT_rows", ROWS)
            if causal:
                dmask_s = constp.tile([128, 4, SLOT], F32)
                nc.sync.dma_start(out=dmask_s, in_=dmask.rearrange(
                    "k p t -> p k t"))
            y_rows_s = ap_.tile([128, NTI, D], F32, tag="y_rows")
            _yr = y_rows.rearrange("(o p) d -> p o d", p=128)
            for _t0 in range(0, NTI, 2):
                nc.sync.dma_start(out=y_rows_s[:, _t0:_t0 + 2, :],
                                  in_=_yr[:, _t0:_t0 + 2, :])
            wk2_s = load_w(wk2, DI, D, "wk2")
            wv2_s = load_w(wv2, DI, D, "wv2")
            bk2_s = load_pbias(bk2, DO, "bk2")
            zT_s = load_split(ap_, zT_full, "zT", S)
            lnb_s = [load_fbias(lnb[i], f"lnb{i}")
                     for i in range(2 if zb3 else 3)]
            if apply_g:
                lng_s = [load_fbias(lng[i], f"lng{i}") for i in range(3)]
            wq2_s = load_w(wq2, DI, D, "wq2")
            bq2_s = load_pbias(bq2, DO, "bq2")
            w1_s = load_w(w1, DI, F, "w1")
            b1_s = load_pbias(b1, NFT, "b1")
            w2_s = load_w(w2, NFT, D, "w2")

            # ---- persistent activations ----
            y1_s = ap_.tile([128, NTI, D], F32, tag="y1")
            y1T_s = ap_.tile([128, DO, ROWS], BF16, tag="y1T")
            y2_s = ap_.tile([128, NTI, D], F32, tag="y2")
            y2T_s = ap_.tile([128, DO, ROWS], BF16, tag="y2T")
            x3_s = ap_.tile([128, NTI, D], F32, tag="x3")
            h1T_s = ap_.tile([128, NFT, ROWS], BF16, tag="h1T")
            k1T_s = ap_.tile([128, DO, S], BF16, tag="k1T")
            v1_s = ap_.tile([128, NKS, D + 2], BF16, tag="v1")
            q1T_s = ap_.tile([128, DO, ROWS], BF16, tag="q1T")
            k2T_s = ap_.tile([128, DO, S], BF16, tag="k2T")
            v2_s = ap_.tile([128, NKS, D + 2], BF16, tag="v2")
            q2T_s = ap_.tile([128, DO, ROWS], BF16, tag="q2T")

            # ones / zero pad columns of V (softmax denominator trick):
            # one strided memset covers all key tiles
            for v_s in (v1_s, v2_s):
                nc.vector.memset(v_s[:, :, D:D + 1], 1.0)
                nc.vector.memset(v_s[:, :, D + 1:D + 2], 0.0)

            # ---- emission helpers ----
            def proj_unit(dst, w_s, b_s, src_T, do, c0, cw):
                """dst[:, do, c0:c0+cw] = (w^T @ src_T)[:, c0:] + b."""
                ps = psMM.tile([128, 512], F32, tag="prj")
                for di in range(DI):
                    nc.tensor.matmul(
                        ps[:, :cw],
                        w_s[:, di, do * 128:(do + 1) * 128],
                        src_T[:, di, c0:c0 + cw],
                        start=(di == 0), stop=(di == DI - 1))
                nc.scalar.activation(
                    out=dst[:, do, c0:c0 + cw], in_=ps[:, :cw],
                    func=AF.Identity, bias=b_s[:, do:do + 1], scale=1.0)

            def projV_unit(dst, wv_s, src_T, c):
                """dst[:, 4c:4c+4, :D] = rows of (X wv) for 4 key tiles."""
                for st in range(4 * c, 4 * c + 4):
                    ps = psMM.tile([128, 512], F32, tag="prj")
                    for di in range(DI):
                        nc.tensor.matmul(
                            ps[:, :D],
                            src_T[:, di, st * 128:(st + 1) * 128],
                            wv_s[:, di, :],
                            start=(di == 0), stop=(di == DI - 1))
                    nc.scalar.activation(
                        out=dst[:, st, :D], in_=ps[:, :D], func=AF.Copy)

            def emit_scores(j, kT_s, qT_s, self_attn):
                """exp(K^T Q + mask) for slot j -> pt tile (bf16)."""
                nks = n_keytiles(j) if self_attn else NKS
                nch = (nks + 3) // 4
                pt = ptp.tile([128, NKS, SLOT], BF16, tag="pt")
                for ch in range(nch):
                    st_ps = psST.tile([128, 4, SLOT], F32, tag="st")
                    for i in range(4):
                        ks = ch * 4 + i
                        for di in range(DO):
                            nc.tensor.matmul(
                                st_ps[:, i, :],
                                kT_s[:, di, ks * KT:(ks + 1) * KT],
                                qT_s[:, di, j * SLOT:(j + 1) * SLOT],
                                start=(di == 0), stop=(di == DO - 1))
                    if self_attn:
                        if causal and ch == nch - 1:
                            nc.vector.tensor_add(out=st_ps, in0=st_ps,
                                                 in1=dmask_s)
                        elif not causal:
                            mk = mkp1.tile([128, 4, SLOT], F32, tag="gmask")
                            nc.sync.dma_start(
                                out=mk,
                                in_=gmask[:, ch * 4:ch * 4 + 4,
                                          j * SLOT:(j + 1) * SLOT])
                            nc.vector.tensor_add(out=st_ps, in0=st_ps,
                                                 in1=mk)
                    nc.scalar.activation(
                        out=pt[:, ch * 4:ch * 4 + 4, :], in_=st_ps,
                        func=AF.Exp)
                return pt, nks

            def emit_pv(j, pt, nks, v_s):
                """P @ V for both 128-row tiles of slot j -> psum pair."""
                o_pair = []
                for th in range(2):
                    o_ps = psPV.tile([128, 512], F32, tag="pv")
                    for ks in range(nks):
                        nc.tensor.matmul(
                            o_ps[:, :D + 2],
                            pt[:, ks, th * 128:(th + 1) * 128],
                            v_s[:, ks, :],
                            start=(ks == 0), stop=(ks == nks - 1))
                    o_pair.append(o_ps)
                return o_pair

            def emit_drain(j, o_pair, resid_s, dst_s):
                """dst = o/denom + resid, one fused DVE op per tile."""
                for th in range(2):
                    ti = 2 * j + th
                    o_ps = o_pair[th]
                    rinv = smallp.tile([128, 1], F32, tag="rinv")
                    nc.vector.reciprocal(out=rinv, in_=o_ps[:, D:D + 1])
                    nc.vector.scalar_tensor_tensor(
                        out=dst_s[:, ti, :], in0=o_ps[:, :D], scalar=rinv,
                        in1=resid_s[:, ti, :], op0=A.mult, op1=A.add)

            def rsqrt2(ve):
                """[128, 2] <- 1/sqrt(ve), DVE-only (no ACT table switch:
                the Ln/Sqrt activation tables don't contain Exp, so using
                them costs a 1.3us ACT_TABLE_LOAD on every switch).

                Fast-inverse-sqrt seed + 1 Newton step; ~2e-3 max rel
                err, well inside the layer's error budget."""
                yi = smallp.tile([128, 2], mybir.dt.int32, tag="rs_i")
                nc.vector.tensor_scalar(out=yi, in0=ve.bitcast(
                    mybir.dt.int32), scalar1=1, scalar2=None,
                    op0=A.logical_shift_right)
                nc.vector.tensor_tensor(out=yi, in0=magic_t, in1=yi,
                                        op=A.subtract)
                y = yi.bitcast(F32)
                h = smallp.tile([128, 2], F32, tag="rs_h")
                nc.vector.tensor_scalar(out=h, in0=ve, scalar1=-0.5,
                                        scalar2=None, op0=A.mult)
                t2 = smallp.tile([128, 2], F32, tag="rs_t")
                nc.vector.tensor_mul(out=t2, in0=y, in1=y)
                nc.vector.tensor_mul(out=t2, in0=h, in1=t2)
                nc.vector.tensor_scalar(out=t2, in0=t2, scalar1=1.5,
                                        scalar2=None, op0=A.add)
                nc.vector.tensor_mul(out=y, in0=y, in1=t2)
                return y

            def emit_ln(x_s, gi, outs, t0, cast_to=None, ti_base=None):
                """outs(ti) = LN(x_s[:, ti, :]) for the pair (t0, t0+1).

                If cast_to is given, a bf16 copy is produced IN PARALLEL
                on GpSimd (not serially after the fp32 path) and PE
                transposes of the pair are emitted into cast_to."""
                mv = smallp.tile([128, 2, 2], F32, tag="ln_mv")
                for i in range(2):
                    stats = smallp.tile([128, 6], F32, tag="ln_st")
                    nc.vector.bn_stats(out=stats, in_=x_s[:, t0 + i, :])
                    nc.vector.bn_aggr(out=mv[:, i, :], in_=stats)
                ve = smallp.tile([128, 2], F32, tag="ln_ve")
                nc.vector.tensor_scalar(out=ve, in0=mv[:, :, 1],
                                        scalar1=EPS, scalar2=None,
                                        op0=A.add)
                rstd = rsqrt2(ve)
                use_b = (gi < 2) or not zb3
                for i in range(2):
                    dst = outs(t0 + i)
                    par = cast_to is not None and use_b
                    tmp = smallp.tile([128, D], F32, tag="ln_t")
                    t_ap = tmp if par else dst
                    nc.vector.tensor_scalar(
                        out=t_ap, in0=x_s[:, t0 + i, :],
                        scalar1=mv[:, i, 0:1],
                        scalar2=rstd[:, i:i + 1],
                        op0=A.subtract,
                        op1=A.mult)
                    if apply_g:
                        nc.vector.tensor_mul(out=t_ap, in0=t_ap,
                                             in1=lng_s[gi])
                    if use_b:
                        nc.vector.tensor_add(out=dst, in0=t_ap,
                                             in1=lnb_s[gi])
                    if cast_to is not None:
                        ti = t0 + i if ti_base is None else ti_base + i
                        sb = smallp.tile([128, D], BF16, tag="tp_b")
                        if use_b:
                            # bf16 copy built in parallel with the fp32 add
                            nc.gpsimd.tensor_add(out=sb, in0=t_ap,
                                                 in1=lnb_s[gi])
                        else:
                            nc.gpsimd.tensor_copy(out=sb, in_=dst)
                        for dh in range(DO):
                            tp = psPV.tile([128, 128], BF16, tag="pv")
                            nc.tensor.transpose(
                                tp, sb[:, dh * 128:(dh + 1) * 128], identb)
                            nc.scalar.activation(
                                out=cast_to[:, dh, ti * 128:(ti + 1) * 128],
                                in_=tp, func=AF.Copy)

            def attn_tail(j, dst_s, dstT_s, gi):
                emit_ln(dst_s, gi, lambda ti: dst_s[:, ti, :], 2 * j,
                        cast_to=dstT_s)

            def emit_ffn1(c):
                """h1T chunk c (256 cols) for all F tiles, relu+bias."""
                cols = slice(c * SLOT, (c + 1) * SLOT)
                for f0 in range(0, NFT, 2):
                    ps = psMM.tile([128, 2, SLOT], F32, tag="prj")
                    for fi in range(2):
                        f = f0 + fi
                        for di in range(DI):
                            nc.tensor.matmul(
                                ps[:, fi, :],
                                w1_s[:, di, f * 128:(f + 1) * 128],
                                y2T_s[:, di, cols],
                                start=(di == 0), stop=(di == DI - 1))
                    for fi in range(2):
                        f = f0 + fi
                        nc.scalar.activation(
                            out=h1T_s[:, f, cols], in_=ps[:, fi, :],
                            func=AF.Relu, bias=b1_s[:, f:f + 1], scale=1.0)

            def emit_ffn2(c):
                """x3 tiles (2c, 2c+1): ffn2 matmul + residual add."""
                for th in range(2):
                    ti = 2 * c + th
                    ps = psMM.tile([128, 512], F32, tag="prj")
                    for f in range(NFT):
                        nc.tensor.matmul(
                            ps[:, :D],
                            h1T_s[:, f, ti * 128:(ti + 1) * 128],
                            w2_s[:, f, :],
                            start=(f == 0), stop=(f == NFT - 1))
                    nc.vector.tensor_add(out=x3_s[:, ti, :], in0=ps[:, :D],
                                         in1=y2_s[:, ti, :])

            def emit_out(p):
                """LN3 + output DMA for tile pair p."""
                t0 = 2 * p
                o_tiles = {}

                def ot(ti):
                    if ti not in o_tiles:
                        t = outp.tile([128, D], F32, tag="out")
                        o_tiles[ti] = t
                    return o_tiles[ti]

                emit_ln(x3_s, 2, ot, t0)
                for ti in (t0, t0 + 1):
                    nc.sync.dma_start(
                        out=out.rearrange("(o p) d -> p o d",
                                          p=128)[:, ti, :],
                        in_=ot(ti))

            # ================= emission schedule ======================
            # -- startup: K1/V1 per chunk as yT chunks land, then Q1 --
            for c in range(NCH):
                for do in range(DO):
                    proj_unit(k1T_s, wk1_s, bk1_s, yT_full_s, do, c * 512,
                              512)
                projV_unit(v1_s, wv1_s, yT_full_s, c)
            for do in range(DO):
                for c0 in (0, 512):
                    proj_unit(q1T_s, wq1_s, bq1_s, yT_rows_s, do, c0, 512)

            # -- self-attention slots, pipelined --
            # fillers: k2/v2 projection units spread across the slots;
            # q2 is projected per 256-col slot chunk as soon as that
            # slot's y1T lands, so only q2(slot3) remains after tail_3
            def q2_unit(s):
                for do in range(DO):
                    proj_unit(q2T_s, wq2_s, bq2_s, y1T_s, do, s * SLOT,
                              SLOT)

            fillers = []
            for c in range(NCH):
                fillers.append(lambda c=c: [
                    proj_unit(k2T_s, wk2_s, bk2_s, zT_s, do, c * 512, 512)
                    for do in range(DO)])
                fillers.append(lambda c=c: projV_unit(v2_s, wv2_s, zT_s, c))
            fillers.append(lambda: q2_unit(0))   # y1T slot0 ready after t1
            fillers.append(lambda: q2_unit(1))
            fillers.append(lambda: q2_unit(2))
            fill_plan = {0: 2, 1: 2, 2: 3, 3: 4}  # units after each slot
            for j in range(NSLOT):
                pt, nks = emit_scores(j, k1T_s, q1T_s, True)
                o_pair = emit_pv(j, pt, nks, v1_s)
                if j > 0:
                    attn_tail(j - 1, y1_s, y1T_s, 0)
                emit_drain(j, o_pair, y_rows_s, y1_s)
                for _ in range(fill_plan[j]):
                    fillers.pop(0)()
            while fillers:
                fillers.pop(0)()
            attn_tail(NSLOT - 1, y1_s, y1T_s, 0)
            q2_unit(3)

            # -- cross-attention slots + FFN + LN3 + out, pipelined --
            for j in range(NSLOT):
                pt, nks = emit_scores(j, k2T_s, q2T_s, False)
                o_pair = emit_pv(j, pt, nks, v2_s)
                if j > 0:
                    attn_tail(j - 1, y2_s, y2T_s, 1)
                emit_drain(j, o_pair, y1_s, y2_s)
                if j >= 2:
                    emit_ffn1(j - 2)
                    emit_ffn2(j - 2)
                if j >= 3:
                    emit_out(j - 3)
            attn_tail(NSLOT - 1, y2_s, y2T_s, 1)
            emit_ffn1(2)
            emit_ffn2(2)
            emit_out(1)
            emit_ffn1(3)
            emit_ffn2(3)
            emit_out(2)
            emit_out(3)

    nc.compile()
    return nc


@functools.lru_cache(maxsize=4)
def _get_program(causal: bool, apply_g: bool, zb3: bool):
    return _build_program(causal, apply_g, zb3)


def _is_causal(mask):
    m = np.asarray(mask)
    if m.shape != (T, S):
        return False
    return bool(np.array_equal(m != 0, np.tril(np.ones((T, S), dtype=bool))))


def _make_dmask(q):
    """Additive diag-chunk mask [4, KT, SLOT] (S^T layout) for parity q."""
    ss = np.arange(KT)[:, None]
    tt = np.arange(SLOT)[None, :]
    out = np.empty((4, KT, SLOT), np.float32)
    for i in range(4):
        out[i] = np.where(128 * i + ss <= 2 * tt + q, 0.0, NEG)
    return out


def _make_gmask(mask, q):
    """General additive mask [KT, NKS, ROWS] (S^T layout) for parity q."""
    rows = np.arange(q, T, 2)                      # owned global rows
    mt = np.where(np.asarray(mask)[rows, :] != 0, 0.0, NEG).astype(np.float32)
    # mt is [ROWS(t), S(s)] -> [s, t] -> [KT, NKS, ROWS]
    return np.ascontiguousarray(
        mt.T.reshape(NKS, KT, ROWS).transpose(1, 0, 2))


def _run(y, Z, target_mask, Wq1, bq1, Wk1, bk1, Wv1, bv1,
         Wq2, bq2, Wk2, bk2, Wv2, bv2, W1, b1, W2, b2,
         g1, be1, g2, be2, g3, be3, trace=False, trace_cores=None):
    y = np.ascontiguousarray(np.asarray(y, np.float32))
    Z = np.ascontiguousarray(np.asarray(Z, np.float32))
    f32 = lambda a: np.asarray(a, np.float32)
    bf = lambda a: np.ascontiguousarray(np.asarray(a, np.float32)
                                        .astype(NPBF16))
    causal = _is_causal(target_mask)
    apply_g = not (np.all(f32(g1) == 1) and np.all(f32(g2) == 1)
                   and np.all(f32(g3) == 1))
    zb3 = bool(np.all(f32(be3) == 0))
    nc = _get_program(causal, apply_g, zb3)

    # host-side bias folding (see module docstring)
    bq2_adj = (f32(bq2) - f32(bv2) @ f32(Wq2)) * ATT_SCALE
    b1_adj = f32(b1) - f32(b2) @ f32(W1)
    lnb0 = f32(be1) + f32(bv2)
    lnb1 = f32(be2) + f32(b2)

    shared = dict(
        wq1=bf(f32(Wq1) * ATT_SCALE), wk1=bf(Wk1), wv1=bf(Wv1),
        wq2=bf(f32(Wq2) * ATT_SCALE), wk2=bf(Wk2), wv2=bf(Wv2),
        w1=bf(W1), w2=bf(W2),
        bq1=f32(bq1) * ATT_SCALE, bk1=f32(bk1),
        bq2=bq2_adj, bk2=f32(bk2), b1=b1_adj,
        lng=np.stack([f32(g1), f32(g2), f32(g3)]),
        lnb=np.stack([lnb0, lnb1, f32(be3)]),
    )
    bv1f = f32(bv1)
    in_maps = []
    for c in range(NCORES):
        b, q = divmod(c, 2)
        rows = y[b, q::2, :]
        m = dict(shared)
        m["y_rows"] = rows + bv1f
        m["yT_rows"] = bf(rows.T)
        m["yT_full"] = bf(y[b].T)
        m["zT_full"] = bf(Z[b].T)
        if causal:
            m["dmask"] = _make_dmask(q)
        else:
            m["gmask"] = _make_gmask(target_mask, q)
        in_maps.append(m)

    res = run_bass_kernel_spmd(nc, in_maps, core_ids=list(range(NCORES)),
                               trace=trace, trace_cores=trace_cores)
    full = np.empty((B, T, D), np.float32)
    for c in range(NCORES):
        b, q = divmod(c, 2)
        full[b, q::2, :] = res.results[c]["out"]
    return full, res


def kernel(**inputs):
    return _run(**inputs)[0]
